# revision 20
# baseline (speedup 1.0000x reference)
"""Gaussian falloff vortex-velocity kernel for Trainium2 (Bass/Tile).

Math per batch element b (single vortex y,x,tau,sig per batch):
    d1 = py - y;  d2 = px - x;  q = d1^2 + d2^2
    s  = tau * exp(-q/sig^2) / sqrt(q)
    out[..., 0] = s * d2;  out[..., 1] = -s * d1

16-bit formulation. The host quantizes the points with a per-batch
zero-point at the vortex (affine quantization): t1 = y - py and
t2 = px - x in f32, rounded to bf16. The quantization error is then
RELATIVE to the distance d, so there is no catastrophic cancellation
near the vortex, and bf16's f32-exponent range makes the pipeline
immune to fp16 denormal flush (q reaches 4e-9 on this data; s reaches
~1e4). Outputs are fp16 (|v| <= tau < 1) and upcast on the host.
Simulated end-to-end error vs the f32 reference: l2 = 3.4e-3,
scale-relative absmax = 8.8e-3 (gate: 2e-2).

Per core: 8 batches, each point-plane [512,512] -> [128, 2048].
On-chip per batch (planes te = t1, to = t2):
    Qe = te*te; Qo = to*to    DVE tt bf16 (2x 16-bit mode, 0.59ns/col)
    q  = Qe + Qo              Pool tt bf16 (gpsimd; only TensorTensor
                              is supported on Pool)
    L  = Ln(q + 1e-30)        ACT  fp16 (eps: q==0 -> s finite, out 0)
    E  = Exp(-q/sig^2+ln tau) ACT  fp16, per-batch scale/bias APs
    R  = Exp(-0.5 * L)        ACT  bf16 = 1/sqrt(q)
                              (ACT Rsqrt itself is blocked in bass)
    s  = E * R                DVE  bf16
    oe = to*s;  oo = te*s     DVE  fp16 out planes
Loads + stores ride the otherwise-idle sync (SP) DMA ring.

Measured engine rates per batch: DVE 5*1212 = 6.1us, ACT 3*2001 =
6.0us, Pool 4.25us, DMA 16MB/core ~ 40us -- near the ~48us roofline.
The 5-stage software pipeline (load / squares / q / Ln+Exp+Exp /
products+store) emits each engine's stream in input-readiness order
(old-item products after new-item squares on DVE) so no engine
head-of-line blocks on a dependency that is still in flight.
"""

import numpy as np
import ml_dtypes

import concourse.bass as bass
import concourse.bacc as bacc
import concourse.mybir as mybir
from concourse.tile import TileContext
from concourse.bass_utils import run_bass_kernel_spmd
from concourse.hw_specs import get_activation_tables

N_CORES = 8
B_PER_CORE = 8          # 64 batches / 8 cores
P = 128                 # SBUF partitions
W = 2048                # per-plane columns per partition (512*512/128)
NCONST = 2              # -1/sig^2, ln(tau)

BF16 = ml_dtypes.bfloat16

_PROGRAM = None


def _pin_act_table_set(arch: str):
    """Make all our activation functions resolve to the single
    `natural_log_exp_and_others` table set. The table-load inserter picks
    the FIRST set containing each function (Exp -> exp_and_others,
    Ln -> natural_log), which thrashes 2 table loads (~2.6us) per batch.
    get_activation_tables() is functools.cached and returns a mutable
    dict of sets; removing our functions from every other set (keeping
    indices intact) makes the combined set the unique first match."""
    AF = mybir.ActivationFunctionType
    try:
        tables = get_activation_tables(arch)
        keep = "natural_log_exp_and_others"
        needed = {AF.Identity, AF.Ln, AF.Exp, AF.Copy}
        if keep not in tables or not needed <= tables[keep]:
            return  # unexpected table layout: skip pinning (correct, slower)
        for name, fns in tables.items():
            if name != keep:
                fns -= needed
    except Exception:
        pass


def _build_program():
    f32 = mybir.dt.float32
    f16 = mybir.dt.float16
    bf16 = mybir.dt.bfloat16
    AF = mybir.ActivationFunctionType
    OP = mybir.AluOpType

    nc = bacc.Bacc(
        "TRN2",
        target_bir_lowering=False,
        debug=False,
        num_devices=N_CORES,
    )
    _pin_act_table_set(nc.m.arch)
    # tin cols: [t1-plane | t2-plane]; tout cols: [v0-plane | v1-plane]
    tin = nc.declare_dram_parameter("tin", [B_PER_CORE * P, 2 * W], bf16, isOutput=False)
    # consts: NCONST per batch + a trailing eps (1e-30) column for Ln's bias
    cst = nc.declare_dram_parameter("consts", [P, NCONST * B_PER_CORE + 1], f32, isOutput=False)
    out = nc.declare_dram_parameter("tout", [B_PER_CORE * P, 2 * W], f16, isOutput=True)

    with TileContext(nc) as tc:
        with (
            tc.tile_pool(name="cpool", bufs=1) as cpool,
            tc.tile_pool(name="tp", bufs=6) as tp,        # T tiles (in planes)
            tc.tile_pool(name="qe", bufs=3) as qe_pool,   # Qb (both squares)
            tc.tile_pool(name="qq", bufs=3) as qq_pool,   # q
            tc.tile_pool(name="lp", bufs=2) as l_pool,
            tc.tile_pool(name="rp", bufs=3) as r_pool,
            tc.tile_pool(name="ep", bufs=3) as e_pool,
            tc.tile_pool(name="sp", bufs=2) as s_pool,
            tc.tile_pool(name="op", bufs=4) as o_pool,    # out planes
        ):
            # Consts first on the sync ring: 8KB, lands ~1us after the ring
            # starts, ahead of the first 1MB T load on the same ring.
            c = cpool.tile([P, NCONST * B_PER_CORE + 1], f32)
            nc.sync.dma_start(c[:], cst[:])
            eps_ap = c[:, NCONST * B_PER_CORE : NCONST * B_PER_CORE + 1]

            # Warm-up activation with no dependencies: walrus inserts the ACT
            # table load (natural_log_exp_and_others) before the first
            # activation; doing it here keeps the load off the critical path.
            w0 = cpool.tile([P, 1], f32)
            nc.vector.memset(w0[:], 1.0)
            nc.scalar.activation(w0[:], w0[:], AF.Exp)

            def cap(b, j):
                return c[:, NCONST * b + j : NCONST * b + j + 1]

            # 5-stage software pipeline over work items (batch col-chunks):
            #   A (step i):   load T(i)                       [SP ring]
            #   B (step i+1): Qe, Qo, q = Qe + Qo             [DVE]
            #   C (step i+2): L, E, R                         [ACT]
            #   D (step i+3): s, oe, oo                       [DVE]
            #   E (step i+4): store O                         [SP ring]
            # GpSimd is deliberately unused: it shares an SBUF port with
            # the vector engine, and a gpsimd tensor op running beside a
            # saturated DVE throttles concurrent DVE tts ~4x (measured).
            # The store is a step later than the products so its trigger's
            # dependency is already resolved when SP reaches it -- otherwise
            # the NEXT load trigger queues behind a blocked store trigger
            # (SP executes its ring in order) and input tiles arrive late.
            # First/last batches split in col-halves to shorten fill/drain.
            items = []
            for b in range(B_PER_CORE):
                if b == 0:  # quarter-split: first compute needs only 256KB of DMA
                    for k in range(4):
                        items.append((b, k * (W // 4), W // 4))
                elif b == B_PER_CORE - 1:
                    items.append((b, 0, W // 2))
                    items.append((b, W // 2, W // 2))
                else:
                    items.append((b, 0, W))
            Ts, Qbs, qs, Rs, Es, Os = {}, {}, {}, {}, {}, {}

            def stage_a(i):
                b, c0, w = items[i]
                rows = slice(b * P, (b + 1) * P)
                T = tp.tile([P, 2 * w], bf16, tag="T")
                if w == W:
                    nc.sync.dma_start(T[:], tin[rows, :])
                else:
                    nc.sync.dma_start(T[:, :w], tin[rows, c0 : c0 + w])
                    nc.sync.dma_start(T[:, w:], tin[rows, W + c0 : W + c0 + w])
                Ts[i] = T

            def stage_b(i):
                b, c0, w = items[i]
                T = Ts[i]
                # DVE and ACT are the only two usable lanes (GpSimd poisons
                # DVE); balance them by giving ACT's Square a ~3/8 slice of
                # the second plane. ACT then carries Sq + L + E + R = 6.8us
                # per batch vs DVE's 6.9us.
                c = (w * 3 // 8 + 63) & ~63
                Qb = qe_pool.tile([P, 2 * w], bf16, tag="Qb")
                q = qq_pool.tile([P, w], bf16, tag="q")
                nc.vector.tensor_tensor(Qb[:, :w], T[:, :w], T[:, :w], OP.mult)
                nc.scalar.activation(Qb[:, w : w + c], T[:, w : w + c], AF.Square)
                nc.vector.tensor_tensor(Qb[:, w + c :], T[:, w + c :], T[:, w + c :], OP.mult)
                nc.vector.tensor_tensor(q[:], Qb[:, :w], Qb[:, w:], OP.add)
                Qbs[i], qs[i] = Qb, q

            def stage_c(i):
                b, c0, w = items[i]
                q = qs[i]
                L = l_pool.tile([P, w], f16, tag="L")
                R = r_pool.tile([P, w], bf16, tag="R")
                E = e_pool.tile([P, w], f16, tag="E")
                # L = ln(q + 1e-30): eps keeps L finite at q==0 so
                # s = E*R stays finite (bf16) and out = 0 * s = 0.
                nc.scalar.activation(L[:], q[:], AF.Ln, bias=eps_ap)
                # E = tau * exp(-q/sig^2)
                nc.scalar.activation(E[:], q[:], AF.Exp, bias=cap(b, 1), scale=cap(b, 0))
                # R = exp(-L/2) = 1/sqrt(q)   (ACT Rsqrt is blocked in bass)
                nc.scalar.activation(R[:], L[:], AF.Exp, scale=-0.5)
                Rs[i], Es[i] = R, E

            def stage_d(i):
                b, c0, w = items[i]
                T, R, E = Ts[i], Rs[i], Es[i]
                te, to = T[:, :w], T[:, w:]
                s = s_pool.tile([P, w], bf16, tag="s")
                O = o_pool.tile([P, 2 * w], f16, tag="O")
                nc.vector.tensor_tensor(s[:], E[:], R[:], OP.mult)
                nc.vector.tensor_tensor(O[:, :w], to, s[:], OP.mult)
                nc.vector.tensor_tensor(O[:, w:], te, s[:], OP.mult)
                Os[i] = O
                del Ts[i], Qbs[i], qs[i], Rs[i], Es[i]

            def stage_e(i):
                b, c0, w = items[i]
                rows = slice(b * P, (b + 1) * P)
                O = Os[i]
                if w == W:
                    nc.sync.dma_start(out[rows, :], O[:])
                else:
                    nc.sync.dma_start(out[rows, c0 : c0 + w], O[:, :w])
                    nc.sync.dma_start(out[rows, W + c0 : W + c0 + w], O[:, w:])
                del Os[i]

            # Emission order per step: loads first (SP), then each engine's
            # stream in input-readiness order -- new-item squares (DVE)
            # before old-item products (DVE) so the products, which wait on
            # ACT results, never head-of-line block independent work.
            NI = len(items)
            for t in range(NI + 4):
                if t < NI:
                    stage_a(t)
                if t >= 4:
                    stage_e(t - 4)
                if 1 <= t < NI + 1:
                    stage_b(t - 1)
                if 2 <= t < NI + 2:
                    stage_c(t - 2)
                if 3 <= t < NI + 3:
                    stage_d(t - 3)

    nc.compile()
    return nc


def _get_program():
    global _PROGRAM
    if _PROGRAM is None:
        _PROGRAM = _build_program()
    return _PROGRAM


def _make_in_maps(vortex_feature, points):
    B = points.shape[0]
    vf = np.asarray(vortex_feature, dtype=np.float32).reshape(B, 6)
    y, x, tau, sig = vf[:, 0], vf[:, 1], vf[:, 2], vf[:, 3]
    sig_c = np.maximum(sig, 1e-35)  # sig==0 -> E=exp(-inf*q)=0 like reference
    ninv = (-1.0 / (sig_c * sig_c)).astype(np.float32)
    with np.errstate(divide="ignore"):
        lnt = np.log(tau).astype(np.float32)  # tau==0 -> -inf -> E=0
    consts = np.stack([ninv, lnt], axis=1)  # [B, 2]
    ncol = NCONST * B_PER_CORE + 1

    pts = np.asarray(points, dtype=np.float32)
    # Affine quantization: subtract the per-batch vortex location in f32,
    # round to bf16. t1 is negated (y - py) so out[...,1] = s * t1.
    t1 = (y[:, None, None] - pts[..., 0]).astype(BF16)
    t2 = (pts[..., 1] - x[:, None, None]).astype(BF16)

    in_maps = []
    for i in range(N_CORES):
        sl = slice(i * B_PER_CORE, (i + 1) * B_PER_CORE)
        tin = np.concatenate(
            [t1[sl].reshape(B_PER_CORE * P, W), t2[sl].reshape(B_PER_CORE * P, W)],
            axis=1,
        )
        crow = np.concatenate(
            [consts[sl].reshape(NCONST * B_PER_CORE), np.float32([1e-30])]
        ).reshape(1, ncol)
        cshard = np.ascontiguousarray(np.broadcast_to(crow, (P, ncol)))
        in_maps.append({"tin": np.ascontiguousarray(tin), "consts": cshard})
    return in_maps


def run(vortex_feature, points, trace=False, tmpdir=None):
    nc = _get_program()
    in_maps = _make_in_maps(vortex_feature, points)
    # The first execution of a freshly-loaded NEFF occasionally hits a
    # transient NRT_EXEC_UNIT_UNRECOVERABLE; a retry reliably succeeds.
    last_err = None
    for _ in range(3):
        try:
            res = run_bass_kernel_spmd(nc, in_maps, list(range(N_CORES)), trace=trace, tmpdir=tmpdir)
            break
        except Exception as err:  # noqa: BLE001
            last_err = err
    else:
        raise last_err
    B, H, Wd, _ = points.shape
    out = np.empty((B, H, Wd, 2), dtype=np.float32)
    for i in range(N_CORES):
        sl = slice(i * B_PER_CORE, (i + 1) * B_PER_CORE)
        o = res.results[i]["tout"].astype(np.float32)
        out[sl, ..., 0] = o[:, :W].reshape(B_PER_CORE, H, Wd)
        out[sl, ..., 1] = o[:, W:].reshape(B_PER_CORE, H, Wd)
    return out, res


def kernel(vortex_feature: np.ndarray, points: np.ndarray) -> np.ndarray:
    out, _ = run(vortex_feature, points, trace=False)
    return out


# revision 23
# speedup vs baseline: 1.0072x; 1.0072x over previous
"""Gaussian falloff vortex-velocity kernel for Trainium2 (Bass/Tile).

Math per batch element b (single vortex y,x,tau,sig per batch):
    d1 = py - y;  d2 = px - x;  q = d1^2 + d2^2
    s  = tau * exp(-q/sig^2) / sqrt(q)
    out[..., 0] = s * d2;  out[..., 1] = -s * d1

16-bit formulation. The host quantizes the points with a per-batch
zero-point at the vortex (affine quantization): t1 = y - py and
t2 = px - x in f32, rounded to bf16. The quantization error is then
RELATIVE to the distance d, so there is no catastrophic cancellation
near the vortex, and bf16's f32-exponent range makes the pipeline
immune to fp16 denormal flush (q reaches 4e-9 on this data; s reaches
~1e4). Outputs are fp16 (|v| <= tau < 1) and upcast on the host.
Simulated end-to-end error vs the f32 reference: l2 = 3.4e-3,
scale-relative absmax = 8.8e-3 (gate: 2e-2).

Per core: 8 batches, each point-plane [512,512] -> [128, 2048].
On-chip per batch (planes te = t1, to = t2):
    Qe = te*te; Qo = to*to    DVE tt bf16 (2x 16-bit mode, 0.59ns/col)
    q  = Qe + Qo              Pool tt bf16 (gpsimd; only TensorTensor
                              is supported on Pool)
    L  = Ln(q + 1e-30)        ACT  fp16 (eps: q==0 -> s finite, out 0)
    E  = Exp(-q/sig^2+ln tau) ACT  fp16, per-batch scale/bias APs
    R  = Exp(-0.5 * L)        ACT  bf16 = 1/sqrt(q)
                              (ACT Rsqrt itself is blocked in bass)
    s  = E * R                DVE  bf16
    oe = to*s;  oo = te*s     DVE  fp16 out planes
Loads + stores ride the otherwise-idle sync (SP) DMA ring.

Measured engine rates per batch: DVE 5*1212 = 6.1us, ACT 3*2001 =
6.0us, Pool 4.25us, DMA 16MB/core ~ 40us -- near the ~48us roofline.
The 5-stage software pipeline (load / squares / q / Ln+Exp+Exp /
products+store) emits each engine's stream in input-readiness order
(old-item products after new-item squares on DVE) so no engine
head-of-line blocks on a dependency that is still in flight.
"""

import numpy as np
import ml_dtypes

import concourse.bass as bass
import concourse.bacc as bacc
import concourse.mybir as mybir
from concourse.tile import TileContext
from concourse.bass_utils import run_bass_kernel_spmd
from concourse.hw_specs import get_activation_tables

N_CORES = 8
B_PER_CORE = 8          # 64 batches / 8 cores
P = 128                 # SBUF partitions
W = 2048                # per-plane columns per partition (512*512/128)
NCONST = 2              # -1/sig^2, ln(tau)

BF16 = ml_dtypes.bfloat16

_PROGRAM = None


def _pin_act_table_set(arch: str):
    """Make all our activation functions resolve to the single
    `natural_log_exp_and_others` table set. The table-load inserter picks
    the FIRST set containing each function (Exp -> exp_and_others,
    Ln -> natural_log), which thrashes 2 table loads (~2.6us) per batch.
    get_activation_tables() is functools.cached and returns a mutable
    dict of sets; removing our functions from every other set (keeping
    indices intact) makes the combined set the unique first match."""
    AF = mybir.ActivationFunctionType
    try:
        tables = get_activation_tables(arch)
        keep = "natural_log_exp_and_others"
        needed = {AF.Identity, AF.Ln, AF.Exp, AF.Copy}
        if keep not in tables or not needed <= tables[keep]:
            return  # unexpected table layout: skip pinning (correct, slower)
        for name, fns in tables.items():
            if name != keep:
                fns -= needed
    except Exception:
        pass


def _build_program():
    f32 = mybir.dt.float32
    f16 = mybir.dt.float16
    bf16 = mybir.dt.bfloat16
    AF = mybir.ActivationFunctionType
    OP = mybir.AluOpType

    nc = bacc.Bacc(
        "TRN2",
        target_bir_lowering=False,
        debug=False,
        num_devices=N_CORES,
    )
    _pin_act_table_set(nc.m.arch)
    # tin cols: [t1-plane | t2-plane]; tout cols: [v0-plane | v1-plane]
    tin = nc.declare_dram_parameter("tin", [B_PER_CORE * P, 2 * W], bf16, isOutput=False)
    # consts: NCONST per batch + a trailing eps (1e-30) column for Ln's bias
    cst = nc.declare_dram_parameter("consts", [P, NCONST * B_PER_CORE + 1], f32, isOutput=False)
    out = nc.declare_dram_parameter("tout", [B_PER_CORE * P, 2 * W], f16, isOutput=True)

    with TileContext(nc) as tc:
        with (
            tc.tile_pool(name="cpool", bufs=1) as cpool,
            tc.tile_pool(name="tp", bufs=6) as tp,        # T tiles (in planes)
            tc.tile_pool(name="qe", bufs=3) as qe_pool,   # Qb (both squares)
            tc.tile_pool(name="qq", bufs=3) as qq_pool,   # q
            tc.tile_pool(name="lp", bufs=2) as l_pool,
            tc.tile_pool(name="rp", bufs=3) as r_pool,
            tc.tile_pool(name="ep", bufs=3) as e_pool,
            tc.tile_pool(name="sp", bufs=2) as s_pool,
            tc.tile_pool(name="op", bufs=4) as o_pool,    # out planes
        ):
            # Consts first on the sync ring: 8KB, lands ~1us after the ring
            # starts, ahead of the first 1MB T load on the same ring.
            c = cpool.tile([P, NCONST * B_PER_CORE + 1], f32)
            nc.sync.dma_start(c[:], cst[:])
            eps_ap = c[:, NCONST * B_PER_CORE : NCONST * B_PER_CORE + 1]

            # Warm-up activation with no dependencies: walrus inserts the ACT
            # table load (natural_log_exp_and_others) before the first
            # activation; doing it here keeps the load off the critical path.
            w0 = cpool.tile([P, 1], f32)
            nc.vector.memset(w0[:], 1.0)
            nc.scalar.activation(w0[:], w0[:], AF.Exp)

            def cap(b, j):
                return c[:, NCONST * b + j : NCONST * b + j + 1]

            # 5-stage software pipeline over work items (batch col-chunks):
            #   A (step i):   load T(i)                       [SP ring]
            #   B (step i+1): Qe, Qo, q = Qe + Qo             [DVE]
            #   C (step i+2): L, E, R                         [ACT]
            #   D (step i+3): s, oe, oo                       [DVE]
            #   E (step i+4): store O                         [SP ring]
            # GpSimd is deliberately unused: it shares an SBUF port with
            # the vector engine, and a gpsimd tensor op running beside a
            # saturated DVE throttles concurrent DVE tts ~4x (measured).
            # The store is a step later than the products so its trigger's
            # dependency is already resolved when SP reaches it -- otherwise
            # the NEXT load trigger queues behind a blocked store trigger
            # (SP executes its ring in order) and input tiles arrive late.
            # First/last batches split in col-halves to shorten fill/drain.
            items = []
            for b in range(B_PER_CORE):
                if b in (0, B_PER_CORE - 1):
                    items.append((b, 0, W // 2))
                    items.append((b, W // 2, W // 2))
                else:
                    items.append((b, 0, W))
            Ts, Qbs, qs, Rs, Es, Os = {}, {}, {}, {}, {}, {}

            def stage_a(i):
                b, c0, w = items[i]
                rows = slice(b * P, (b + 1) * P)
                T = tp.tile([P, 2 * w], bf16, tag="T")
                if w == W:
                    nc.sync.dma_start(T[:], tin[rows, :])
                else:
                    nc.sync.dma_start(T[:, :w], tin[rows, c0 : c0 + w])
                    nc.sync.dma_start(T[:, w:], tin[rows, W + c0 : W + c0 + w])
                Ts[i] = T

            def stage_b(i):
                b, c0, w = items[i]
                T = Ts[i]
                # DVE and ACT are the only two usable lanes (GpSimd poisons
                # DVE); balance them by giving ACT's Square a 1/4 slice of
                # the second plane. ACT then carries Sq + L + E + R ~= 6.6us
                # per batch vs DVE's ~6.9us.
                c = w // 4
                Qb = qe_pool.tile([P, 2 * w], bf16, tag="Qb")
                q = qq_pool.tile([P, w], bf16, tag="q")
                if i in (5, 6):
                    # A/B probe: one merged 4096-col square. If the DVE 2x
                    # 16-bit mode holds above 2048 elements (earlier 1x
                    # observation was confounded by GpSimd contention),
                    # the next revision merges everywhere.
                    nc.vector.tensor_tensor(Qb[:], T[:], T[:], OP.mult)
                else:
                    nc.vector.tensor_tensor(Qb[:, :w], T[:, :w], T[:, :w], OP.mult)
                    nc.scalar.activation(Qb[:, w : w + c], T[:, w : w + c], AF.Square)
                    nc.vector.tensor_tensor(Qb[:, w + c :], T[:, w + c :], T[:, w + c :], OP.mult)
                nc.vector.tensor_tensor(q[:], Qb[:, :w], Qb[:, w:], OP.add)
                Qbs[i], qs[i] = Qb, q

            def stage_c(i):
                b, c0, w = items[i]
                q = qs[i]
                L = l_pool.tile([P, w], f16, tag="L")
                R = r_pool.tile([P, w], bf16, tag="R")
                E = e_pool.tile([P, w], f16, tag="E")
                # L = ln(q + 1e-30): eps keeps L finite at q==0 so
                # s = E*R stays finite (bf16) and out = 0 * s = 0.
                nc.scalar.activation(L[:], q[:], AF.Ln, bias=eps_ap)
                # E = tau * exp(-q/sig^2)
                nc.scalar.activation(E[:], q[:], AF.Exp, bias=cap(b, 1), scale=cap(b, 0))
                # R = exp(-L/2) = 1/sqrt(q)   (ACT Rsqrt is blocked in bass)
                nc.scalar.activation(R[:], L[:], AF.Exp, scale=-0.5)
                Rs[i], Es[i] = R, E

            def stage_d(i):
                b, c0, w = items[i]
                T, R, E = Ts[i], Rs[i], Es[i]
                te, to = T[:, :w], T[:, w:]
                s = s_pool.tile([P, w], bf16, tag="s")
                O = o_pool.tile([P, 2 * w], f16, tag="O")
                nc.vector.tensor_tensor(s[:], E[:], R[:], OP.mult)
                nc.vector.tensor_tensor(O[:, :w], to, s[:], OP.mult)
                nc.vector.tensor_tensor(O[:, w:], te, s[:], OP.mult)
                Os[i] = O
                del Ts[i], Qbs[i], qs[i], Rs[i], Es[i]

            def stage_e(i):
                b, c0, w = items[i]
                rows = slice(b * P, (b + 1) * P)
                O = Os[i]
                # Store triggers ride the GpSimd DGE ring: the gpsimd CPU is
                # compute-idle (its tensor ops would contend with DVE for the
                # SBUF port, but DGE descriptor generation does not), and a
                # separate ring keeps store triggers from delaying loads.
                if w == W:
                    nc.gpsimd.dma_start(out[rows, :], O[:])
                else:
                    nc.gpsimd.dma_start(out[rows, c0 : c0 + w], O[:, :w])
                    nc.gpsimd.dma_start(out[rows, W + c0 : W + c0 + w], O[:, w:])
                del Os[i]

            # Emission order per step: loads first (SP), then each engine's
            # stream in input-readiness order -- new-item squares (DVE)
            # before old-item products (DVE) so the products, which wait on
            # ACT results, never head-of-line block independent work.
            NI = len(items)
            for t in range(NI + 4):
                if t < NI:
                    stage_a(t)
                if t >= 4:
                    stage_e(t - 4)
                if 1 <= t < NI + 1:
                    stage_b(t - 1)
                if 2 <= t < NI + 2:
                    stage_c(t - 2)
                if 3 <= t < NI + 3:
                    stage_d(t - 3)

    nc.compile()
    return nc


def _get_program():
    global _PROGRAM
    if _PROGRAM is None:
        _PROGRAM = _build_program()
    return _PROGRAM


def _make_in_maps(vortex_feature, points):
    B = points.shape[0]
    vf = np.asarray(vortex_feature, dtype=np.float32).reshape(B, 6)
    y, x, tau, sig = vf[:, 0], vf[:, 1], vf[:, 2], vf[:, 3]
    sig_c = np.maximum(sig, 1e-35)  # sig==0 -> E=exp(-inf*q)=0 like reference
    ninv = (-1.0 / (sig_c * sig_c)).astype(np.float32)
    with np.errstate(divide="ignore"):
        lnt = np.log(tau).astype(np.float32)  # tau==0 -> -inf -> E=0
    consts = np.stack([ninv, lnt], axis=1)  # [B, 2]
    ncol = NCONST * B_PER_CORE + 1

    pts = np.asarray(points, dtype=np.float32)
    # Affine quantization: subtract the per-batch vortex location in f32,
    # round to bf16. t1 is negated (y - py) so out[...,1] = s * t1.
    t1 = (y[:, None, None] - pts[..., 0]).astype(BF16)
    t2 = (pts[..., 1] - x[:, None, None]).astype(BF16)

    in_maps = []
    for i in range(N_CORES):
        sl = slice(i * B_PER_CORE, (i + 1) * B_PER_CORE)
        tin = np.concatenate(
            [t1[sl].reshape(B_PER_CORE * P, W), t2[sl].reshape(B_PER_CORE * P, W)],
            axis=1,
        )
        crow = np.concatenate(
            [consts[sl].reshape(NCONST * B_PER_CORE), np.float32([1e-30])]
        ).reshape(1, ncol)
        cshard = np.ascontiguousarray(np.broadcast_to(crow, (P, ncol)))
        in_maps.append({"tin": np.ascontiguousarray(tin), "consts": cshard})
    return in_maps


def run(vortex_feature, points, trace=False, tmpdir=None):
    nc = _get_program()
    in_maps = _make_in_maps(vortex_feature, points)
    # The first execution of a freshly-loaded NEFF occasionally hits a
    # transient NRT_EXEC_UNIT_UNRECOVERABLE; a retry reliably succeeds.
    last_err = None
    for _ in range(3):
        try:
            res = run_bass_kernel_spmd(nc, in_maps, list(range(N_CORES)), trace=trace, tmpdir=tmpdir)
            break
        except Exception as err:  # noqa: BLE001
            last_err = err
    else:
        raise last_err
    B, H, Wd, _ = points.shape
    out = np.empty((B, H, Wd, 2), dtype=np.float32)
    for i in range(N_CORES):
        sl = slice(i * B_PER_CORE, (i + 1) * B_PER_CORE)
        o = res.results[i]["tout"].astype(np.float32)
        out[sl, ..., 0] = o[:, :W].reshape(B_PER_CORE, H, Wd)
        out[sl, ..., 1] = o[:, W:].reshape(B_PER_CORE, H, Wd)
    return out, res


def kernel(vortex_feature: np.ndarray, points: np.ndarray) -> np.ndarray:
    out, _ = run(vortex_feature, points, trace=False)
    return out
